# revision 18
# baseline (speedup 1.0000x reference)
"""Chamfer distance (L2, squared) Bass kernel for Trainium2 — windowed-NN.

Problem: xyz1 (4, 8192, 3), xyz2 (4, 8192, 3) float32.
  d2[b, n, m] = ||xyz1[b,n] - xyz2[b,m]||^2
  out = mean_n(min_m d2) + mean_m(min_n d2)   (scalar, float32)

Strategy (exact, not approximate):
  Host z-sorts each cloud. Each 128-query chunk only compares against a
  window of W z-consecutive candidates centered on its rank range (host
  gathers the window coords). Any candidate OUTSIDE the window is at
  |dz| >= gap, so if the windowed min <= gap^2 the window min IS the
  global min. The host flags the (few hundred of 65536) queries failing
  that bound and recomputes them exactly in numpy. Device work per core
  drops from 4096x8192 to 64 chunks x 128 x W distances, and BOTH
  reduction directions become free-axis minima (queries always sit on
  PSUM partitions) — no partition reduction anywhere.

Sharding: 8 cores = (batch b in 0..3) x (side: dist1 | dist2). Each core:
  64 chunks; chunk j = queries sorted[128j:128j+128] vs its gathered
  window [W]. One bf16 matmul (16-row hi/lo feature decomposition, exact
  to ~1e-6) -> PSUM [128, W] -> min over free axis -> mins[128, 64].

Consumption routes per 4-chunk group (tunable engine balance):
  alpha: ScalarE evacuates PSUM->SBUF fp16; DVE tensor_tensor_reduce
         (min of the two window halves + min-reduce) -> column.
  beta:  DVE tensor_tensor_reduce directly on the two PSUM halves.
  gamma: ScalarE evac; GpSimd tensor_tensor folds 512->256; DVE TTR 256.
Features are built on the HOST (hi/lo bf16 split) and DMA'd in at prep.
"""

import numpy as np
import ml_dtypes

import concourse.bass as bass
import concourse.tile as tile
from concourse import bacc, mybir
from concourse.bass_utils import run_bass_kernel_spmd

B, N, M = 4, 8192, 8192
NCORES = 8

W = 256  # candidate window per 128-query chunk
NCH = 64  # chunks per core (8192 queries / 128)
G = 8  # chunks per PSUM group
NGRP = NCH // G
NF = 24  # feature rows (three-level hi/mid/lo bf16 decomposition)

# Route mix (groups): alpha = ScalarE evac to fp16 + DVE TT-min tree +
# reduce; beta = DVE tensor_reduce directly from PSUM. (TensorTensorReduce
# and other custom DVE ops crash this runtime; gpsimd has no elementwise
# min. So ScalarE + DVE standard ops are the only consumers.)
N_BETA = 0
N_DMA = 0  # groups evacuated by DMA (fp32) + DVE reduce
B_AT_END = 1  # place beta/dma groups at the end of the pass
TREE_D = 2  # TT-min halving levels before the final tensor_reduce

F32 = mybir.dt.float32
BF16 = mybir.dt.bfloat16
FP16 = mybir.dt.float16
BIG = 3.0e38
BF = ml_dtypes.bfloat16

MIN = mybir.AluOpType.min
AXIS_X = mybir.AxisListType.X


def _build_body(tc, qf_t, wf_t, mins_t, repeat):
    nc = tc.nc

    persist = tc.alloc_tile_pool(name="persist", bufs=1)
    fq = persist.tile([NF, NCH * 128], BF16)
    fw = persist.tile([NF, NCH * W], BF16)
    negacc = persist.tile([128, NCH], F32)
    outt = persist.tile([128, NCH], F32)

    nc.sync.dma_start(fq[:], qf_t.ap())
    nc.sync.dma_start(fw[:], wf_t.ap())

    aux = tc.alloc_tile_pool(name="aux", bufs=1)
    ps_pool = tc.alloc_tile_pool(name="ps_pool", bufs=2, space="PSUM")

    counts = {"a": NGRP - N_BETA - N_DMA, "b": N_BETA, "d": N_DMA}
    if B_AT_END:
        routes = ["a"] * counts["a"] + ["b"] * N_BETA + ["d"] * N_DMA
    else:
        # Proportional interleave of routes.
        routes = []
        cred = dict.fromkeys(counts, 0.0)
        left = dict(counts)
        for _ in range(NGRP):
            for k in counts:
                if left[k]:
                    cred[k] += counts[k] / NGRP
            best = max((k for k in counts if left[k]), key=lambda k: cred[k])
            cred[best] -= 1.0
            left[best] -= 1
            routes.append(best)

    def one_pass():
        nalpha = 0
        for g, route in enumerate(routes):
            ps = ps_pool.tile([128, G, W], F32, tag="ps", name="ps")
            for j in range(G):
                ch = g * G + j
                nc.tensor.matmul(
                    ps[:, j, :],
                    fq[:, 128 * ch : 128 * (ch + 1)],
                    fw[:, W * ch : W * (ch + 1)],
                    start=True,
                    stop=True,
                )
            if route == "b":
                # Direct free-axis min from PSUM on DVE.
                nc.vector.tensor_reduce(
                    negacc[:, g * G : (g + 1) * G], ps[:], axis=AXIS_X, op=MIN
                )
            else:
                # ScalarE evacuates fp16 into one half of a pair tile; after
                # the odd group of each pair, DVE runs ONE TT-min tree over
                # both groups (4D APs) — halves DVE op-count overhead.
                half = nalpha % 2
                nalpha += 1
                if half == 0:
                    gtp = aux.tile(
                        [128, 2, G, W], FP16, tag="gtp", name="gtp", bufs=2
                    )
                    one_pass.gtp = gtp
                else:
                    gtp = one_pass.gtp
                nc.scalar.copy(gtp[:, half], ps[:])
                last_alpha = "a" not in routes[g + 1 :]
                if half == 1 or last_alpha:
                    nh = half + 1  # groups present in this pair tile
                    cur = gtp[:, 0:nh]
                    w = W
                    for d in range(TREE_D):
                        w //= 2
                        nxt = aux.tile(
                            [128, nh, G, w], FP16, tag=f"tr{d}", name=f"tr{d}", bufs=2
                        )
                        nc.vector.tensor_tensor(
                            nxt[:], cur[:, :, :, 0:w], cur[:, :, :, w : 2 * w], op=MIN
                        )
                        cur = nxt[:]
                    nc.vector.tensor_reduce(
                        negacc[:, (g - nh + 1) * G : (g + 1) * G],
                        cur,
                        axis=AXIS_X,
                        op=MIN,
                    )

    if repeat == 1:
        one_pass()
    else:
        with tc.For_i(0, repeat, 1):
            one_pass()

    ps_pool.release()

    # Tail: clamp d2 >= 0 (reference clamps before the min; clamp is
    # monotone so clamping the min is identical), then DMA out.
    nc.vector.tensor_scalar_max(outt[:], negacc[:], 0.0)
    nc.sync.dma_start(mins_t.ap(), outt[:])

    aux.release()
    persist.release()


def build_nc(repeat=1):
    nc = bacc.Bacc("TRN2", target_bir_lowering=False, debug=False, num_devices=NCORES)
    qf_t = nc.dram_tensor("qf", [NF, NCH * 128], BF16, kind="ExternalInput")
    wf_t = nc.dram_tensor("wf", [NF, NCH * W], BF16, kind="ExternalInput")
    mins_t = nc.dram_tensor("mins", [128, NCH], F32, kind="ExternalOutput")
    with tile.TileContext(nc) as tc:
        _build_body(tc, qf_t, wf_t, mins_t, repeat)
    nc.compile()
    return nc


_NC_CACHE = {}


def get_nc(repeat=1):
    if repeat not in _NC_CACHE:
        _NC_CACHE[repeat] = build_nc(repeat)
    return _NC_CACHE[repeat]


def _split3(x):
    """f32/f64 array -> (hi, mid, lo) bf16 with hi+mid+lo ~= x (~2^-27 rel)."""
    x = x.astype(np.float64)
    hi = x.astype(BF)
    r = x - hi.astype(np.float64)
    mid = r.astype(BF)
    lo = (r - mid.astype(np.float64)).astype(BF)
    return hi, mid, lo


def _features(pts, scale, kind):
    """pts [L, 3] f32 -> [24, L] bf16 feature rows (3-level decomposition).

    q-column . w-column = scale*(q.c) + |q|^2 + |c|^2 with ~1e-6 abs error:
    products kept: yh*xh + yh*xm + ym*xh + yh*xl + yl*xh + ym*xm (rows 0-17),
    norms as three bf16 levels paired against ones (rows 18-23).
    """
    L = pts.shape[0]
    y = pts.astype(np.float64) * scale
    yh, ym, yl = _split3(y)
    n = (pts.astype(np.float64) ** 2).sum(1)
    nh, nm, nl = _split3(n)
    f = np.empty((NF, L), BF)
    one = np.ones(L, BF)
    if kind == "q":
        blocks = [yh, yh, ym, yh, yl, ym]
    else:
        blocks = [yh, ym, yh, yl, yh, ym]
    for i, blk in enumerate(blocks):
        f[3 * i : 3 * i + 3] = blk.T
    if kind == "q":
        f[18], f[19], f[20] = nh, nm, nl
        f[21] = f[22] = f[23] = one
    else:
        f[18] = f[19] = f[20] = one
        f[21], f[22], f[23] = nh, nm, nl
    return f


_CTX = None


def make_in_maps(xyz1, xyz2):
    """Sort, window, featurize. Caches fixup context in _CTX."""
    global _CTX
    xyz1 = np.asarray(xyz1, np.float32)
    xyz2 = np.asarray(xyz2, np.float32)
    starts = np.clip(np.arange(NCH) * 128 + 64 - W // 2, 0, M - W)
    in_maps = []
    ctx = []
    for b in range(B):
        s1 = xyz1[b][np.argsort(xyz1[b, :, 2], kind="stable")]
        s2 = xyz2[b][np.argsort(xyz2[b, :, 2], kind="stable")]
        for side, (q, c) in enumerate(((s1, s2), (s2, s1))):
            win = np.concatenate([c[a : a + W] for a in starts], 0)
            in_maps.append(
                {
                    "qf": np.ascontiguousarray(_features(q, -2.0, "q")),
                    "wf": np.ascontiguousarray(_features(win, 1.0, "w")),
                }
            )
            ctx.append((q, c, side))
    _CTX = (starts, ctx)
    return in_maps


def combine(results):
    starts, ctx = _CTX
    tot = [0.0, 0.0]  # [dist1 sum, dist2 sum]
    for r, (q, c, side) in zip(results, ctx):
        mins = r["mins"].T.reshape(-1).astype(np.float64)  # sorted-query order
        # Exactness check: excluded candidates are at |dz| >= gap, so a
        # windowed min <= gap^2 is the true global min. Flag the rest
        # (with margin covering fp16 evac + bf16 feature rounding).
        cz = c[:, 2]
        qz = q[:, 2]
        gap = np.full(N, np.inf)
        a = np.repeat(starts, 128)
        lmask = a > 0
        gap[lmask] = qz[lmask] - cz[np.maximum(a - 1, 0)][lmask]
        rmask = a + W < M
        np.minimum(
            gap, np.where(rmask, cz[np.minimum(a + W, M - 1)] - qz, np.inf), out=gap
        )
        # Margin: fp16 evac is value-relative (2^-11), the 3-level bf16
        # feature decomposition is ~1e-6 abs; 1e-3 rel + 5e-5 abs covers
        # both with ~2x slack without over-flagging.
        thr = np.maximum(gap, 0.0) ** 2
        bad = mins > thr * (1.0 - 1e-3) - 5e-5
        if bad.any():
            qb = q[bad].astype(np.float64)
            cd = c.astype(np.float64)
            d2 = (
                (qb**2).sum(1)[:, None]
                + (cd**2).sum(1)[None, :]
                - 2.0 * qb @ cd.T
            )
            mins[bad] = np.maximum(d2.min(1), 0.0)
        tot[side] += mins.sum()
    return np.float32(tot[0] / (B * N) + tot[1] / (B * M))


def kernel(xyz1, xyz2):
    in_maps = make_in_maps(xyz1, xyz2)
    nc = get_nc()
    res = run_bass_kernel_spmd(nc, in_maps, core_ids=list(range(NCORES)))
    return combine(res.results)


if __name__ == "__main__":
    rng = np.random.default_rng(0)
    a = rng.standard_normal((B, N, 3), dtype=np.float32)
    b = rng.standard_normal((B, M, 3), dtype=np.float32)
    print("kernel:", kernel(a, b))


# revision 30
# speedup vs baseline: 1.1712x; 1.1712x over previous
"""Chamfer distance (L2, squared) Bass kernel for Trainium2 — windowed-NN.

Problem: xyz1 (4, 8192, 3), xyz2 (4, 8192, 3) float32.
  d2[b, n, m] = ||xyz1[b,n] - xyz2[b,m]||^2
  out = mean_n(min_m d2) + mean_m(min_n d2)   (scalar, float32)

Strategy (exact, not approximate):
  Host z-sorts each cloud. Each 128-query chunk only compares against a
  window of W z-consecutive candidates centered on its rank range (host
  gathers the window coords). Any candidate OUTSIDE the window is at
  |dz| >= gap, so if the windowed min <= gap^2 the window min IS the
  global min. The host flags the (few hundred of 65536) queries failing
  that bound and recomputes them exactly in numpy. Device work per core
  drops from 4096x8192 to 64 chunks x 128 x W distances, and BOTH
  reduction directions become free-axis minima (queries always sit on
  PSUM partitions) — no partition reduction anywhere.

Sharding: 8 cores = (batch b in 0..3) x (side: dist1 | dist2). Each core:
  64 chunks; chunk j = queries sorted[128j:128j+128] vs its gathered
  window [W]. One bf16 matmul (16-row hi/lo feature decomposition, exact
  to ~1e-6) -> PSUM [128, W] -> min over free axis -> mins[128, 64].

Consumption routes per 4-chunk group (tunable engine balance):
  alpha: ScalarE evacuates PSUM->SBUF fp16; DVE tensor_tensor_reduce
         (min of the two window halves + min-reduce) -> column.
  beta:  DVE tensor_tensor_reduce directly on the two PSUM halves.
  gamma: ScalarE evac; GpSimd tensor_tensor folds 512->256; DVE TTR 256.
Features are built on the HOST (hi/lo bf16 split) and DMA'd in at prep.
"""

import numpy as np
import ml_dtypes

import concourse.bass as bass
import concourse.tile as tile
from concourse import bacc, mybir
from concourse.bass_utils import run_bass_kernel_spmd

B, N, M = 4, 8192, 8192
NCORES = 8

W = 256  # candidate window per 128-query chunk
NCH = 64  # chunks per core (8192 queries / 128)
G = 8  # chunks per PSUM group
NGRP = NCH // G
NF = 24  # feature rows (three-level hi/mid/lo bf16 decomposition)

# Route mix (groups): alpha = ScalarE evac to fp16 + DVE TT-min tree +
# reduce; beta = DVE tensor_reduce directly from PSUM. (TensorTensorReduce
# and other custom DVE ops crash this runtime; gpsimd has no elementwise
# min. So ScalarE + DVE standard ops are the only consumers.)
N_BETA = 0
B_AT_END = 1  # place beta groups at the end of the pass
TREE_D = 2  # TT-min halving levels before the final tensor_reduce
PAIR_TREE = 0  # fuse the DVE tree across pairs of groups
CONSUME = "full"  # "full" | "evac" (no tree) | "mmonly" (ablation timing)
ROW_TILE = 1  # K=24 <= 32: pack 4 concurrent matmuls into PE row bands

F32 = mybir.dt.float32
BF16 = mybir.dt.bfloat16
FP16 = mybir.dt.float16
BIG = 3.0e38
BF = ml_dtypes.bfloat16

MIN = mybir.AluOpType.min
AXIS_X = mybir.AxisListType.X


def _build_body(tc, qf_t, wf_t, mins_t, repeat):
    nc = tc.nc

    persist = tc.alloc_tile_pool(name="persist", bufs=1)
    if ROW_TILE:
        # chunk c lives in PE row band b(c) = 32*((c%8)//2), column block
        # i(c) = (c//8)*2 + c%2; bands stream concurrently on the PE.
        fq = persist.tile([128, (NCH // 4) * 128], BF16)
        fw = persist.tile([128, (NCH // 4) * W], BF16)
    else:
        fq = persist.tile([NF, NCH * 128], BF16)
        fw = persist.tile([NF, NCH * W], BF16)
    negacc = persist.tile([128, NCH], F32)
    outt = persist.tile([128, NCH], F32)

    nc.sync.dma_start(fq[:], qf_t.ap())
    nc.sync.dma_start(fw[:], wf_t.ap())

    aux = tc.alloc_tile_pool(name="aux", bufs=1)
    ps_pool = tc.alloc_tile_pool(name="ps_pool", bufs=2, space="PSUM")

    counts = {"a": NGRP - N_BETA, "b": N_BETA}
    if B_AT_END:
        routes = ["a"] * counts["a"] + ["b"] * N_BETA
    else:
        # Proportional interleave of routes.
        routes = []
        cred = dict.fromkeys(counts, 0.0)
        left = dict(counts)
        for _ in range(NGRP):
            for k in counts:
                if left[k]:
                    cred[k] += counts[k] / NGRP
            best = max((k for k in counts if left[k]), key=lambda k: cred[k])
            cred[best] -= 1.0
            left[best] -= 1
            routes.append(best)

    def one_pass():
        nalpha = 0
        for g, route in enumerate(routes):
            ps = ps_pool.tile([128, G, W], F32, tag="ps", name="ps")
            if ROW_TILE:
                # Emit in quads of distinct bands AND distinct PSUM banks:
                # j order 0,2,4,6 then 1,3,5,7; band = j//2.
                for j in [x for x in range(0, G, 2)] + [x for x in range(1, G, 2)]:
                    ch = g * G + j
                    band = 32 * ((j % 8) // 2)
                    idx = (ch // 8) * 2 + ch % 2
                    nc.tensor.matmul(
                        ps[:, j, :],
                        fq[band : band + NF, 128 * idx : 128 * (idx + 1)],
                        fw[band : band + NF, W * idx : W * (idx + 1)],
                        start=True,
                        stop=True,
                        tile_position=(band, 0),
                    )
            else:
                for j in range(G):
                    ch = g * G + j
                    nc.tensor.matmul(
                        ps[:, j, :],
                        fq[:, 128 * ch : 128 * (ch + 1)],
                        fw[:, W * ch : W * (ch + 1)],
                        start=True,
                        stop=True,
                    )
            if CONSUME == "mmonly":
                nc.vector.tensor_reduce(
                    negacc[:, g * G : (g + 1) * G], ps[:, :, 0:1], axis=AXIS_X, op=MIN
                )
            elif CONSUME == "evac":
                gt = aux.tile([128, G, W], FP16, tag="gte", name="gte", bufs=3)
                nc.scalar.copy(gt[:], ps[:])
                nc.vector.tensor_reduce(
                    negacc[:, g * G : (g + 1) * G], gt[:, :, 0:1], axis=AXIS_X, op=MIN
                )
            elif route == "b":
                # Direct free-axis min from PSUM on DVE.
                nc.vector.tensor_reduce(
                    negacc[:, g * G : (g + 1) * G], ps[:], axis=AXIS_X, op=MIN
                )
            else:
                # ScalarE evacuates fp16 into one half of a pair tile; after
                # the odd group of each pair, DVE runs ONE TT-min tree over
                # both groups (4D APs) — halves DVE op-count overhead.
                half = (nalpha % 2) if PAIR_TREE else 0
                nalpha += 1 if PAIR_TREE else 2
                if half == 0:
                    gtp = aux.tile(
                        [128, 2, G, W], FP16, tag="gtp", name="gtp", bufs=2
                    )
                    one_pass.gtp = gtp
                else:
                    gtp = one_pass.gtp
                nc.scalar.copy(gtp[:, half], ps[:])
                last_alpha = "a" not in routes[g + 1 :]
                if half == 1 or last_alpha or not PAIR_TREE:
                    nh = half + 1  # groups present in this pair tile
                    cur = gtp[:, 0:nh]
                    w = W
                    for d in range(TREE_D):
                        w //= 2
                        nxt = aux.tile(
                            [128, nh, G, w], FP16, tag=f"tr{d}", name=f"tr{d}", bufs=2
                        )
                        nc.vector.tensor_tensor(
                            nxt[:], cur[:, :, :, 0:w], cur[:, :, :, w : 2 * w], op=MIN
                        )
                        cur = nxt[:]
                    nc.vector.tensor_reduce(
                        negacc[:, (g - nh + 1) * G : (g + 1) * G],
                        cur,
                        axis=AXIS_X,
                        op=MIN,
                    )

    if repeat == 1:
        one_pass()
    else:
        with tc.For_i(0, repeat, 1):
            one_pass()

    ps_pool.release()

    # Tail: clamp d2 >= 0 (reference clamps before the min; clamp is
    # monotone so clamping the min is identical), then DMA out.
    nc.vector.tensor_scalar_max(outt[:], negacc[:], 0.0)
    nc.sync.dma_start(mins_t.ap(), outt[:])

    aux.release()
    persist.release()


def build_nc(repeat=1):
    nc = bacc.Bacc("TRN2", target_bir_lowering=False, debug=False, num_devices=NCORES)
    if ROW_TILE:
        qf_t = nc.dram_tensor("qf", [128, (NCH // 4) * 128], BF16, kind="ExternalInput")
        wf_t = nc.dram_tensor("wf", [128, (NCH // 4) * W], BF16, kind="ExternalInput")
    else:
        qf_t = nc.dram_tensor("qf", [NF, NCH * 128], BF16, kind="ExternalInput")
        wf_t = nc.dram_tensor("wf", [NF, NCH * W], BF16, kind="ExternalInput")
    mins_t = nc.dram_tensor("mins", [128, NCH], F32, kind="ExternalOutput")
    with tile.TileContext(nc) as tc:
        _build_body(tc, qf_t, wf_t, mins_t, repeat)
    nc.compile()
    return nc


_NC_CACHE = {}


def get_nc(repeat=1):
    if repeat not in _NC_CACHE:
        _NC_CACHE[repeat] = build_nc(repeat)
    return _NC_CACHE[repeat]


def _split3(x):
    """f32/f64 array -> (hi, mid, lo) bf16 with hi+mid+lo ~= x (~2^-27 rel)."""
    x = x.astype(np.float64)
    hi = x.astype(BF)
    r = x - hi.astype(np.float64)
    mid = r.astype(BF)
    lo = (r - mid.astype(np.float64)).astype(BF)
    return hi, mid, lo


def _features(pts, scale, kind):
    """pts [L, 3] f32 -> [24, L] bf16 feature rows (3-level decomposition).

    q-column . w-column = scale*(q.c) + |q|^2 + |c|^2 with ~1e-6 abs error:
    products kept: yh*xh + yh*xm + ym*xh + yh*xl + yl*xh + ym*xm (rows 0-17),
    norms as three bf16 levels paired against ones (rows 18-23).
    """
    L = pts.shape[0]
    y = pts.astype(np.float64) * scale
    yh, ym, yl = _split3(y)
    n = (pts.astype(np.float64) ** 2).sum(1)
    nh, nm, nl = _split3(n)
    f = np.empty((NF, L), BF)
    one = np.ones(L, BF)
    if kind == "q":
        blocks = [yh, yh, ym, yh, yl, ym]
    else:
        blocks = [yh, ym, yh, yl, yh, ym]
    for i, blk in enumerate(blocks):
        f[3 * i : 3 * i + 3] = blk.T
    if kind == "q":
        f[18], f[19], f[20] = nh, nm, nl
        f[21] = f[22] = f[23] = one
    else:
        f[18] = f[19] = f[20] = one
        f[21], f[22], f[23] = nh, nm, nl
    return f


_CTX = None


def make_in_maps(xyz1, xyz2):
    """Sort, window, featurize. Caches fixup context in _CTX."""
    global _CTX
    xyz1 = np.asarray(xyz1, np.float32)
    xyz2 = np.asarray(xyz2, np.float32)
    starts = np.clip(np.arange(NCH) * 128 + 64 - W // 2, 0, M - W)
    in_maps = []
    ctx = []
    for b in range(B):
        s1 = xyz1[b][np.argsort(xyz1[b, :, 2], kind="stable")]
        s2 = xyz2[b][np.argsort(xyz2[b, :, 2], kind="stable")]
        for side, (q, c) in enumerate(((s1, s2), (s2, s1))):
            win = np.concatenate([c[a : a + W] for a in starts], 0)
            qf = _features(q, -2.0, "q")
            wf = _features(win, 1.0, "w")
            if ROW_TILE:
                qb = np.zeros((128, (NCH // 4) * 128), BF)
                wb = np.zeros((128, (NCH // 4) * W), BF)
                for ch in range(NCH):
                    b = 32 * ((ch % 8) // 2)
                    i = (ch // 8) * 2 + ch % 2
                    qb[b : b + NF, 128 * i : 128 * (i + 1)] = qf[
                        :, 128 * ch : 128 * (ch + 1)
                    ]
                    wb[b : b + NF, W * i : W * (i + 1)] = wf[
                        :, W * ch : W * (ch + 1)
                    ]
                qf, wf = qb, wb
            in_maps.append(
                {
                    "qf": np.ascontiguousarray(qf),
                    "wf": np.ascontiguousarray(wf),
                }
            )
            ctx.append((q, c, side))
    _CTX = (starts, ctx)
    return in_maps


def combine(results):
    starts, ctx = _CTX
    tot = [0.0, 0.0]  # [dist1 sum, dist2 sum]
    for r, (q, c, side) in zip(results, ctx):
        mins = r["mins"].T.reshape(-1).astype(np.float64)  # sorted-query order
        # Exactness check: excluded candidates are at |dz| >= gap, so a
        # windowed min <= gap^2 is the true global min. Flag the rest
        # (with margin covering fp16 evac + bf16 feature rounding).
        cz = c[:, 2]
        qz = q[:, 2]
        gap = np.full(N, np.inf)
        a = np.repeat(starts, 128)
        lmask = a > 0
        gap[lmask] = qz[lmask] - cz[np.maximum(a - 1, 0)][lmask]
        rmask = a + W < M
        np.minimum(
            gap, np.where(rmask, cz[np.minimum(a + W, M - 1)] - qz, np.inf), out=gap
        )
        # Margin: fp16 evac is value-relative (2^-11), the 3-level bf16
        # feature decomposition is ~1e-6 abs; 1e-3 rel + 5e-5 abs covers
        # both with ~2x slack without over-flagging.
        thr = np.maximum(gap, 0.0) ** 2
        bad = mins > thr * (1.0 - 1e-3) - 5e-5
        if bad.any():
            qb = q[bad].astype(np.float64)
            cd = c.astype(np.float64)
            d2 = (
                (qb**2).sum(1)[:, None]
                + (cd**2).sum(1)[None, :]
                - 2.0 * qb @ cd.T
            )
            mins[bad] = np.maximum(d2.min(1), 0.0)
        tot[side] += mins.sum()
    return np.float32(tot[0] / (B * N) + tot[1] / (B * M))


def kernel(xyz1, xyz2):
    in_maps = make_in_maps(xyz1, xyz2)
    nc = get_nc()
    res = run_bass_kernel_spmd(nc, in_maps, core_ids=list(range(NCORES)))
    return combine(res.results)


if __name__ == "__main__":
    rng = np.random.default_rng(0)
    a = rng.standard_normal((B, N, 3), dtype=np.float32)
    b = rng.standard_normal((B, M, 3), dtype=np.float32)
    print("kernel:", kernel(a, b))
